# revision 1
# baseline (speedup 1.0000x reference)
"""GAU (Gated Attention Unit) kernel for 8 Trainium2 NeuronCores.

Full inputs in, full output out.  Sharding: data-parallel over batch (4)
x sequence-parallel over output rows (2) = 8 shards, one per core.  Each
core computes k/v for its batch's full sequence and attention outputs for
its half of the rows.  The second-half core receives its tokens rotated by
half the sequence so the device program is identical on every core; the
position-dependent Toeplitz bias is supplied per-core as a precomputed
band table (T[i,j] = g(i-j), a RoPE relative-position identity).
"""

import numpy as np
import ml_dtypes
from contextlib import ExitStack

import concourse.bass as bass
import concourse.bacc as bacc
import concourse.tile as tile
from concourse import mybir
from concourse.bass_utils import run_bass_kernel_spmd
from concourse.masks import make_identity

BF16 = mybir.dt.bfloat16
F32 = mybir.dt.float32
FP8 = mybir.dt.float8e4
NPBF16 = ml_dtypes.bfloat16

DIM = 512
SH = 128      # shared (qk) dim
EXP = 1024    # expansion dim
PROJ = 2 * EXP + SH  # 2176
LN_EPS = 1e-3
FC = DIM // 128      # feature chunks (4)
PC = PROJ // 128     # proj chunks (17)
NBLK = 512           # n-block width for attention


def _plan(T):
    """Static loop/table geometry for sequence length T."""
    TOWN = T // 2
    MT = T // 128
    NB = TOWN // NBLK
    mhalf = MT // 2
    s0 = lambda mt, nb: nb * NBLK - mt * 128 + T
    sA = [s0(mt, nb) for mt in range(mhalf) for nb in range(NB)]
    sB = [s0(mt, nb) for mt in range(mhalf, MT) for nb in range(NB)]
    baseA, widthA = min(sA), max(sA) + NBLK - min(sA)
    baseB, widthB = min(sB), max(sB) + NBLK - min(sB)
    return dict(T=T, TOWN=TOWN, MT=MT, NB=NB, mhalf=mhalf,
                baseA=baseA, widthA=widthA, baseB=baseB, widthB=widthB)


def _toeplitz_band(a, b, T):
    """g[d], d in [-(T-1), T-1], with T_mat[i, j] = g[i - j + T - 1].

    rope_rows(v, n)[i] = R(theta*i) v pairwise; <R(ti)a, R(tj)b> depends
    only on i-j:  g(d) = sum_f (a1*b1 + a2*b2) cos(d*th_f)
                             + (a1*b2 - a2*b1) sin(d*th_f).
    """
    half = T // 2
    a = np.asarray(a, np.float64)
    b = np.asarray(b, np.float64)
    inv = 10000.0 ** (-(np.arange(half, dtype=np.float64) / half))
    c = a[:half] * b[:half] + a[half:] * b[half:]
    s = a[:half] * b[half:] - a[half:] * b[:half]
    d = np.arange(-(T - 1), T, dtype=np.float64)
    ang = d[:, None] * inv[None, :]
    g = np.cos(ang) @ c + np.sin(ang) @ s
    return g.astype(np.float64)


def _band_tables(g, plan, delta_b):
    """HA/HB tables: H[r, s] = g((s + base) - r - T + delta)."""
    T = plan["T"]
    r = np.arange(128)[:, None]

    def tab(base, width, delta):
        s = np.arange(width)[None, :]
        arg = (s + base) - r - T + delta
        assert arg.min() >= -(T - 1) and arg.max() <= T - 1, (arg.min(), arg.max())
        return g[arg + T - 1].astype(NPBF16)

    ha = tab(plan["baseA"], plan["widthA"], 0)
    hb = tab(plan["baseB"], plan["widthB"], delta_b)
    return ha, hb


def _build_kernel_body(ctx, tc, io, plan, silu_native, spec_beta0,
                       b1v_bc, b2_bc):
    nc = tc.nc
    T, TOWN, MT, NB = plan["T"], plan["TOWN"], plan["MT"], plan["NB"]
    mhalf = plan["mhalf"]
    NTB = T // NBLK       # token blocks of 512 over full seq
    NTBO = TOWN // NBLK   # token blocks over own rows

    SiluF = mybir.ActivationFunctionType.Silu
    SigF = mybir.ActivationFunctionType.Sigmoid
    SqrtF = mybir.ActivationFunctionType.Sqrt
    SquareF = mybir.ActivationFunctionType.Square
    ReluF = mybir.ActivationFunctionType.Relu
    Alu = mybir.AluOpType

    consts = ctx.enter_context(tc.tile_pool(name="consts", bufs=1))
    big32 = ctx.enter_context(tc.tile_pool(name="big32", bufs=1))
    stpool = ctx.enter_context(tc.tile_pool(name="stpool", bufs=3))
    tpose = ctx.enter_context(tc.tile_pool(name="tpose", bufs=2))
    acts = ctx.enter_context(tc.tile_pool(name="acts", bufs=1))
    gpool = ctx.enter_context(tc.tile_pool(name="gpool", bufs=2))
    xstream = ctx.enter_context(tc.tile_pool(name="xstream", bufs=3))
    stats = ctx.enter_context(tc.tile_pool(name="stats", bufs=4))
    sgpool = ctx.enter_context(tc.tile_pool(name="sgpool", bufs=2))
    ostream = ctx.enter_context(tc.tile_pool(name="ostream", bufs=2))
    dram = ctx.enter_context(tc.tile_pool(name="dram", bufs=1, space="DRAM"))
    psmm = ctx.enter_context(
        tc.tile_pool(name="psmm", bufs=2, space=bass.MemorySpace.PSUM))
    psattn = ctx.enter_context(
        tc.tile_pool(name="psattn", bufs=4, space=bass.MemorySpace.PSUM))

    # ---- constants in SBUF ----
    w1_sb = consts.tile([128, FC, PROJ], FP8)
    nc.sync.dma_start(w1_sb, io["w1"].rearrange("(c p) n -> p c n", p=128))
    w2_sb = consts.tile([128, EXP // 128, DIM], FP8)
    nc.sync.dma_start(w2_sb, io["w2"].rearrange("(c p) n -> p c n", p=128))
    b1t_sb = consts.tile([128, PC], F32)
    nc.sync.dma_start(b1t_sb, io["b1t"])
    qkp_sb = consts.tile([128, 4], F32)
    nc.sync.dma_start(qkp_sb, io["qkp"])
    ha_sb = consts.tile([128, plan["widthA"]], BF16)
    nc.sync.dma_start(ha_sb, io["ha"])
    hb_sb = consts.tile([128, plan["widthB"]], BF16)
    nc.sync.dma_start(hb_sb, io["hb"])
    ident = consts.tile([128, 128], BF16)
    make_identity(nc, ident)
    eps_t = consts.tile([128, 1], F32)
    nc.vector.memset(eps_t, LN_EPS)
    if b1v_bc is not None:
        b1v_sb = consts.tile([128, EXP], F32)
        nc.sync.dma_start(b1v_sb, io["b1v"].to_broadcast((128, EXP)))
    if b2_bc is not None:
        b2_sb = consts.tile([128, DIM], F32)
        nc.sync.dma_start(b2_sb, io["b2"].to_broadcast((128, DIM)))

    x_ap = io["x"]
    y_ap = io["y"]

    # ---- phase 0: layernorm (natural) -> bf16 scratch -> DMA-transpose
    # -> fp8 cast, split into two token halves so proj1 on half 0 overlaps
    # the LayerNorm of half 1.
    TH = T // 2
    MTH = MT // 2

    def ln_half(h2, xn_sc_h, xnT_h):
        for lt in range(MTH):
            mt = h2 * MTH + lt
            xt = xstream.tile([128, DIM], F32, tag="xin")
            nc.sync.dma_start(xt, x_ap[mt * 128:(mt + 1) * 128, :])
            st6 = stats.tile([128, 6], F32)
            nc.vector.bn_stats(st6, xt)
            mv = stats.tile([128, 2], F32)
            nc.vector.bn_aggr(mv, st6)
            rstd = stats.tile([128, 1], F32)
            nc.scalar.activation(rstd, mv[:, 1:2], SqrtF, bias=eps_t,
                                 scale=1.0)
            nc.vector.reciprocal(out=rstd, in_=rstd)
            xn = xstream.tile([128, DIM], BF16, tag="xn")
            nc.vector.tensor_scalar(out=xn, in0=xt, scalar1=mv[:, 0:1],
                                    scalar2=rstd,
                                    op0=Alu.subtract, op1=Alu.mult)
            nc.sync.dma_start(xn_sc_h[lt * 128:(lt + 1) * 128, :], xn)
        # transpose (2-byte-only DMA mode) + fp8 cast per f-chunk
        for fc in range(FC):
            xtb = tpose.tile([128, TH], BF16, tag="xtb")
            nc.sync.dma_start(xtb, xn_sc_h[:, fc * 128:(fc + 1) * 128],
                              transpose=True)
            nc.vector.tensor_copy(xnT_h[:, fc, :], xtb)

    xn_sc0 = dram.tile([TH, DIM], BF16)
    xn_sc1 = dram.tile([TH, DIM], BF16)
    xnT0 = big32.tile([128, FC, TH], FP8, tag="xnT0")
    xnT1 = big32.tile([128, FC, TH], FP8, tag="xnT1")
    xnT_h = (xnT0, xnT1)

    def xnT_sl(c, t0, t1):
        """xnT[:, 2c:2c+2, t0:t1] across the two half-tiles (never straddles)."""
        h2 = 0 if t1 <= TH else 1
        assert (t0 >= TH) == (h2 == 1)
        base = h2 * TH
        return xnT_h[h2][:, 2 * c:2 * c + 2, t0 - base:t1 - base]

    # W1 is host-scaled by 32 for fp8; undo inside the activation
    W1S = 1.0 / 32.0

    def silu_from_psum(out_ap, ps, bias_col):
        if silu_native:
            if bias_col is None:
                nc.scalar.activation(out_ap, ps, SiluF, scale=W1S)
            else:
                nc.scalar.activation(out_ap, ps, SiluF, bias=bias_col,
                                     scale=W1S)
        else:
            # sim-only decomposition: silu(z) = z * sigmoid(z), z = ps*W1S + b1
            sg = sgpool.tile([128, out_ap.shape[-1]], BF16, tag="sg")
            z = sgpool.tile([128, out_ap.shape[-1]], F32, tag="sz")
            if bias_col is None:
                nc.vector.tensor_scalar_mul(out=z, in0=ps, scalar1=W1S)
            else:
                nc.vector.tensor_scalar(out=z, in0=ps, scalar1=W1S,
                                        scalar2=bias_col,
                                        op0=Alu.mult, op1=Alu.add)
            nc.scalar.activation(sg, z, SigF)
            nc.vector.tensor_mul(out_ap, z, sg)

    # ---- phase 1: proj1 (all DoubleRow fp8), interleaved with the LN
    # halves: half-0 matmuls are emitted right after half-0's transpose so
    # they overlap half-1's LayerNorm.
    # fp8: v is only consumed as the DoubleRow lhsT of the attention matmul
    v_sb = acts.tile([128, MT, EXP], FP8)
    uT_sb = acts.tile([128, EXP // 128, TOWN], BF16)
    baseT = acts.tile([128, T], BF16)
    FP2 = FC // 2  # f-chunk pairs for DoubleRow
    DR = mybir.MatmulPerfMode.DoubleRow

    def v_tiles(mt_range):
        for mt in mt_range:
            ps = psmm.tile([128, 2, 512], F32, tag="ps")
            for eb in range(EXP // 512):
                for c in range(FP2):
                    nc.tensor.matmul(
                        ps[:, eb, :],
                        xnT_sl(c, mt * 128, (mt + 1) * 128),
                        w1_sb[:, 2 * c:2 * c + 2,
                              EXP + eb * 512:EXP + (eb + 1) * 512],
                        start=(c == 0), stop=(c == FP2 - 1), perf_mode=DR)
            if b1v_bc is not None:
                tmp = stats.tile([128, EXP], F32, tag="vbias")
                nc.vector.tensor_add(tmp, ps, b1v_sb)
                silu_from_psum(v_sb[:, mt, :], tmp, None)
            else:
                silu_from_psum(v_sb[:, mt, :], ps, None)

    def ub_tiles(out_ap, colk, tb_list, tb_base):
        # out_ap[*, (tb - tb_base)*512 ...] = silu(xn @ W1[:, colk*128:...])T
        for i in range(0, len(tb_list), 2):
            pair = tb_list[i:i + 2]
            ps = psmm.tile([128, 2, 512], F32, tag="ps")
            for j, tb in enumerate(pair):
                for c in range(FP2):
                    nc.tensor.matmul(
                        ps[:, j, :],
                        w1_sb[:, 2 * c:2 * c + 2, colk * 128:(colk + 1) * 128],
                        xnT_sl(c, tb * 512, (tb + 1) * 512),
                        start=(c == 0), stop=(c == FP2 - 1), perf_mode=DR)
            o0 = (pair[0] - tb_base) * 512
            silu_from_psum(out_ap[:, o0:o0 + len(pair) * 512],
                           ps[:, :len(pair), :], b1t_sb[:, colk:colk + 1])

    ln_half(0, xn_sc0, xnT0)
    ln_half(1, xn_sc1, xnT1)
    HTB = TH // 512  # 512-token blocks per half

    # half-0 consumers first (overlap half-1 LN): v, all of u, base half 0
    v_tiles(range(MTH))
    for pb in range(EXP // 128):
        ub_tiles(uT_sb[:, pb, :], pb, list(range(NTBO)), 0)
    ub_tiles(baseT, 2 * EXP // 128, list(range(HTB)), 0)
    # half-1 consumers
    v_tiles(range(MTH, MT))
    ub_tiles(baseT[:, TH:], 2 * EXP // 128, list(range(HTB, NTB)), HTB)

    # ---- phase 1c: q/k offset-scale ----
    # qT = baseT[:, :TOWN] * qkp[:,0] + qkp[:,1]; spec_beta0 folds both
    # gammas (and the 1/T qk scale) into the q side so kT = baseT as-is.
    qT = acts.tile([128, TOWN], BF16)
    nc.vector.tensor_scalar(out=qT, in0=baseT[:, :TOWN],
                            scalar1=qkp_sb[:, 0:1], scalar2=qkp_sb[:, 1:2],
                            op0=Alu.mult, op1=Alu.add)
    if not spec_beta0:
        nc.vector.tensor_scalar(out=baseT, in0=baseT,
                                scalar1=qkp_sb[:, 2:3], scalar2=qkp_sb[:, 3:4],
                                op0=Alu.mult, op1=Alu.add)
    kT = baseT

    # ---- phase 2/3: attention + gate + proj2, per n-block ----
    MP = MT // 2  # DoubleRow m-pairs
    for nb in range(NB):
        # sT[ki, t, j, n] = relu2 score for m-token (2t+j)*128+ki, col n.
        # [128, 2, NBLK] slices feed the DoubleRow rhs directly.
        sT = stpool.tile([128, MP, 2, NBLK], FP8, tag="sT")
        for t in range(MP):
            ps = psmm.tile([128, 2, NBLK], F32, tag="ps")
            for j in range(2):
                mt = 2 * t + j
                s0 = nb * NBLK - mt * 128 + T
                if mt < mhalf:
                    hsl = ha_sb[:, s0 - plan["baseA"]:
                                s0 - plan["baseA"] + NBLK]
                else:
                    hsl = hb_sb[:, s0 - plan["baseB"]:
                                s0 - plan["baseB"] + NBLK]
                nc.tensor.matmul(ps[:, j, :], ident, hsl,
                                 start=True, stop=False)
                nc.tensor.matmul(ps[:, j, :], kT[:, mt * 128:(mt + 1) * 128],
                                 qT[:, nb * NBLK:(nb + 1) * NBLK],
                                 start=False, stop=True)
            # relu(x)^2 over the m-tile pair: DVE max(x,0) PSUM->SBUF,
            # then ACT square at x32 (a single STT reading ps twice is
            # rejected by neuronx-cc).  The x1024 score scaling keeps sT
            # out of e4m3's denormal range; compensated in proj2's epilogue.
            zr = sgpool.tile([128, 2, NBLK], BF16, tag="sg")
            nc.vector.tensor_scalar_max(out=zr, in0=ps, scalar1=0.0)
            nc.scalar.activation(sT[:, t, :, :], zr, SquareF, scale=32.0)

        gT = gpool.tile([128, EXP // 128, NBLK], FP8, tag="gT")
        for wave in range(2):
            pas = []
            for e4 in range(4):
                pa = psattn.tile([128, NBLK], F32, tag="pa")
                pas.append(pa)
            for t in range(MP):
                for e4 in range(4):
                    ec = wave * 4 + e4
                    nc.tensor.matmul(
                        pas[e4],
                        v_sb[:, 2 * t:2 * t + 2, ec * 128:(ec + 1) * 128],
                        sT[:, t, :, :],
                        start=(t == 0), stop=(t == MP - 1),
                        perf_mode=mybir.MatmulPerfMode.DoubleRow)
            for e4 in range(4):
                ec = wave * 4 + e4
                # rescale by 2^-5 so |gT| stays inside fp8-e4m3 range
                # (psum carries the x1024 score scaling)
                nc.vector.scalar_tensor_tensor(
                    out=gT[:, ec, :], in0=pas[e4], scalar=2.0 ** -5,
                    in1=uT_sb[:, ec, nb * NBLK:(nb + 1) * NBLK],
                    op0=Alu.mult, op1=Alu.mult)

        EP2 = EXP // 256  # e-chunk pairs
        for nt2 in range(0, NBLK // 128, 2):
            psy = psmm.tile([128, 2, DIM], F32, tag="ps")
            for j in range(2):
                nt = nt2 + j
                for c in range(EP2):
                    nc.tensor.matmul(
                        psy[:, j, :],
                        gT[:, 2 * c:2 * c + 2, nt * 128:(nt + 1) * 128],
                        w2_sb[:, 2 * c:2 * c + 2, :],
                        start=(c == 0), stop=(c == EP2 - 1), perf_mode=DR)
            for j in range(2):
                rows = nb * NBLK + (nt2 + j) * 128
                xs = ostream.tile([128, DIM], F32, tag="xs")
                nc.sync.dma_start(xs, x_ap[rows:rows + 128, :])
                ys = ostream.tile([128, DIM], F32, tag="ys")
                # psum carries 1024 (scores) * 2^-5 (gT) * 32 (W2) = 2^10
                nc.vector.scalar_tensor_tensor(
                    out=ys, in0=psy[:, j, :], scalar=2.0 ** -10, in1=xs,
                    op0=Alu.mult, op1=Alu.add)
                if b2_bc is not None:
                    nc.vector.tensor_add(ys, ys, b2_sb)
                nc.sync.dma_start(y_ap[rows:rows + 128, :], ys)


_PROG_CACHE = {}


def _get_program(T, silu_native, spec_beta0, with_b1v, with_b2, repeats=1):
    key = (T, silu_native, spec_beta0, with_b1v, with_b2, repeats)
    if key in _PROG_CACHE:
        return _PROG_CACHE[key]
    plan = _plan(T)
    nc = bacc.Bacc("TRN2", target_bir_lowering=False, debug=False)
    io = {
        "x": nc.dram_tensor("x", [T, DIM], F32, kind="ExternalInput").ap(),
        "w1": nc.dram_tensor("w1", [DIM, PROJ], FP8, kind="ExternalInput").ap(),
        "w2": nc.dram_tensor("w2", [EXP, DIM], FP8, kind="ExternalInput").ap(),
        "b1t": nc.dram_tensor("b1t", [128, PC], F32, kind="ExternalInput").ap(),
        "qkp": nc.dram_tensor("qkp", [128, 4], F32, kind="ExternalInput").ap(),
        "ha": nc.dram_tensor("ha", [128, plan["widthA"]], BF16,
                             kind="ExternalInput").ap(),
        "hb": nc.dram_tensor("hb", [128, plan["widthB"]], BF16,
                             kind="ExternalInput").ap(),
        "y": nc.dram_tensor("y", [plan["TOWN"], DIM], F32,
                            kind="ExternalOutput").ap(),
    }
    if with_b1v:
        io["b1v"] = nc.dram_tensor("b1v", [1, EXP], F32,
                                   kind="ExternalInput").ap()
    if with_b2:
        io["b2"] = nc.dram_tensor("b2", [1, DIM], F32,
                                  kind="ExternalInput").ap()
    with tile.TileContext(nc) as tc:
        for _ in range(repeats):
            with ExitStack() as ctx:
                _build_kernel_body(ctx, tc, io, plan, silu_native, spec_beta0,
                                   "b1v" if with_b1v else None,
                                   "b2" if with_b2 else None)
    nc.compile()
    _PROG_CACHE[key] = (nc, plan)
    return nc, plan


def prepare_in_maps(x, ln_gamma, ln_beta, W1, b1, W2, b2, a, b, gamma, beta,
                    silu_native=True, repeats=1):
    """Host-side prep: fold LN affine + qk scale into weights, build the
    Toeplitz band tables, shard per core.  Returns (nc, plan, in_maps, B)."""
    x = np.asarray(x, np.float32)
    B, T, _ = x.shape
    W1 = np.asarray(W1, np.float64)
    W1eff = np.asarray(ln_gamma, np.float64)[:, None] * W1
    b1eff = np.asarray(ln_beta, np.float64) @ W1 + np.asarray(b1, np.float64)
    # fp8 weights, host-scaled by 32 into e4m3's normal range; the kernel
    # multiplies proj1 psums by 1/32 inside the silu activation, and folds
    # W2's 32 together with the relu^2-score x1024 into a 2^-15 epilogue.
    NPFP8 = ml_dtypes.float8_e4m3
    w1_bf = (W1eff.astype(np.float32) * 32.0).astype(NPFP8)
    w2_bf = (np.asarray(W2, np.float32) * 32.0).astype(NPFP8)
    b1t = np.ascontiguousarray(
        b1eff.astype(np.float32).reshape(PC, 128).T)

    gamma = np.asarray(gamma, np.float64)
    beta = np.asarray(beta, np.float64)
    spec_beta0 = bool(np.all(beta == 0.0))
    qkp = np.zeros((128, 4), np.float32)
    if spec_beta0:
        qkp[:, 0] = (gamma[0] * gamma[1] / T).astype(np.float32)
    else:
        qkp[:, 0] = (gamma[0] / T).astype(np.float32)
        qkp[:, 1] = (beta[0] / T).astype(np.float32)
        qkp[:, 2] = gamma[1].astype(np.float32)
        qkp[:, 3] = beta[1].astype(np.float32)

    b1v = np.asarray(b1, np.float32)[EXP:2 * EXP]
    with_b1v = bool(np.any(b1v != 0.0))
    b2 = np.asarray(b2, np.float32)
    with_b2 = bool(np.any(b2 != 0.0))

    nc, plan = _get_program(T, silu_native, spec_beta0, with_b1v, with_b2,
                            repeats=repeats)

    g = _toeplitz_band(a, b, T)
    ha0, hb0 = _band_tables(g, plan, 0)      # first-half cores
    _, hb1 = _band_tables(g, plan, T)        # second-half cores

    in_maps = []
    for core in range(2 * B):
        bidx, h = core // 2, core % 2
        if h == 0:
            xc = x[bidx]
        else:
            xc = np.concatenate([x[bidx, T // 2:], x[bidx, :T // 2]], axis=0)
        m = {"x": np.ascontiguousarray(xc), "w1": w1_bf, "w2": w2_bf,
             "b1t": b1t, "qkp": qkp, "ha": ha0, "hb": hb0 if h == 0 else hb1}
        if with_b1v:
            m["b1v"] = b1v.reshape(1, EXP)
        if with_b2:
            m["b2"] = b2.reshape(1, DIM)
        in_maps.append(m)
    return nc, plan, in_maps, B


def kernel(x, ln_gamma, ln_beta, W1, b1, W2, b2, a, b, gamma, beta):
    x = np.asarray(x, np.float32)
    B, T, D = x.shape
    nc, plan, in_maps, _ = prepare_in_maps(
        x, ln_gamma, ln_beta, W1, b1, W2, b2, a, b, gamma, beta)
    res = run_bass_kernel_spmd(nc, in_maps, list(range(2 * B)))
    out = np.empty((B, T, D), np.float32)
    TOWN = T // 2
    for core in range(2 * B):
        bidx, h = core // 2, core % 2
        out[bidx, h * TOWN:(h + 1) * TOWN] = res.results[core]["y"]
    return out

